# revision 22
# baseline (speedup 1.0000x reference)
"""Trainium2 Bass kernel for nn_Block_56650618634972.

Math: reference = relu(AFFINE(relu(BN1(dwconv3x3(x)))))  where AFFINE is the
composition of 8 butterfly stages + per-stage BNs — all linear over the
256-channel axis — folded on host into a single 256x256 matrix M + bias.

Device work per core (batch-sharded, 4 images each):
  1. x arrives host-padded in a (58x58) zero-ringed layout, so every DMA is a
     full-rate contiguous transfer straight into SBUF.
  2. depthwise 3x3 conv: 9 diagonal-matrix matmuls (f32r) accumulating into
     PSUM, reading shifted windows of the padded tile.
  3. conv epilogue on ScalarE: relu(psum + beta1) -> y (SBUF, f32r)
  4. butterfly: dense 256x256 matmul (f32r, 2x2 blocks of 128)
  5. epilogue on VectorE: relu(psum + bias) -> out (SBUF, f32) -> DMA out

DMA queues: nc.sync carries the x-in stream; nc.scalar (the other HWDGE ring)
carries params + output stores (their issue cost on ACT is tiny).
"""

import numpy as np

import concourse.bass as bass
import concourse.mybir as mybir
import concourse.tile as tile
from concourse import bacc
from concourse.bass_utils import run_bass_kernel_spmd

N_CORES = 8
IMGS = 4            # images per core (32 / 8)
C = 256
H = W = 56
L = H * W           # 3136
HP = H + 2          # 58
WP = W + 2          # 58
PADL = HP * WP      # 3364
EPS = 1e-5
CONV_CHUNK_ROWS = 8                     # 8 rows x 56 = 448 cols <= 512 (1 PSUM bank)
CONV_CHUNK = CONV_CHUNK_ROWS * W        # 448
N_CONV_CHUNKS = H // CONV_CHUNK_ROWS    # 7
BTF_CHUNKS = [(i * 512, min(512, L - i * 512)) for i in range((L + 511) // 512)]

F32 = mybir.dt.float32
F32R = mybir.dt.float32r
RELU = mybir.ActivationFunctionType.Relu
ADD = mybir.AluOpType.add
MAX = mybir.AluOpType.max

_compiled = {}


def _build(imgs=IMGS):
    nc = bacc.Bacc("TRN2", target_bir_lowering=False, debug=False,
                   num_devices=N_CORES)
    x_d = nc.dram_tensor("x", (IMGS, 2, 128, HP, WP), F32R, kind="ExternalInput")
    diag_d = nc.dram_tensor("diag", (128, 18, 128), F32R, kind="ExternalInput")
    mt_d = nc.dram_tensor("mt", (128, 4, 128), F32R, kind="ExternalInput")
    cbias_d = nc.dram_tensor("cbias", (128, 2), F32, kind="ExternalInput")
    obias_d = nc.dram_tensor("obias", (128, 2), F32, kind="ExternalInput")
    w9c_d = nc.dram_tensor("w9c", (128, 18), F32, kind="ExternalInput")
    out_d = nc.dram_tensor("out", (IMGS, C, H, W), F32, kind="ExternalOutput")

    x_v = x_d.ap().rearrange("n t c a b -> n t c a b")
    out_v = out_d.ap().rearrange("n (t c) h w -> n c t (h w)", t=2)

    from contextlib import ExitStack
    with tile.TileContext(nc) as tc, ExitStack() as es:
        consts = es.enter_context(tc.tile_pool(name="consts", bufs=1))
        xp_pool = es.enter_context(tc.tile_pool(name="xppool", bufs=4))
        y_pool = es.enter_context(tc.tile_pool(name="ypool", bufs=2))
        o_pool = es.enter_context(tc.tile_pool(name="opool", bufs=2))
        cps_pool = es.enter_context(tc.tile_pool(name="cps", bufs=3, space="PSUM"))
        bps_pool = es.enter_context(tc.tile_pool(name="bps", bufs=2, space="PSUM"))

        # ---- constants: diag halves on sync ring; first xp piece + small
        # params on the scalar ring so both rings fill in parallel ----
        # first-matmul-critical bytes split across BOTH rings in parallel:
        # diag taps for ct0 on sync; xp00 pieces on scalar (behind tiny params)
        diag_sb = consts.tile([128, 18, 128], F32R, name="diag_sb", tag="diag_sb")
        nc.sync.dma_start(out=diag_sb[:, 0:9], in_=diag_d.ap()[:, 0:9])
        w9c_sb = consts.tile([128, 18], F32, name="w9c_sb", tag="w9c_sb")
        nc.scalar.dma_start(out=w9c_sb, in_=w9c_d.ap())
        cbias_sb = consts.tile([128, 2], F32, name="cbias_sb", tag="cbias_sb")
        nc.scalar.dma_start(out=cbias_sb, in_=cbias_d.ap())
        xp00 = xp_pool.tile([128, HP, WP], F32R, tag="xp", name="xp0_0")
        nc.scalar.dma_start(out=xp00[:, 0:35], in_=x_v[0, 0][:, 0:35])
        nc.scalar.dma_start(out=xp00[:, 35:], in_=x_v[0, 0][:, 35:])
        nc.sync.dma_start(out=diag_sb[:, 9:18], in_=diag_d.ap()[:, 9:18])
        obias_sb = consts.tile([128, 2], F32, name="obias_sb", tag="obias_sb")
        nc.scalar.dma_start(out=obias_sb, in_=obias_d.ap())
        mt_sb = consts.tile([128, 4, 128], F32R, name="mt_sb", tag="mt_sb")
        nc.scalar.dma_start(out=mt_sb, in_=mt_d.ap())

        # conv chunking: 16 rows (896 cols = 2 PSUM banks) + ragged last 8 rows
        CCHUNKS = [(0, 16), (16, 16), (32, 16), (48, 8)]
        PE_TAPS_BY_CT = (7, 6)

        for n in range(imgs):
            xps = []
            for ct in range(2):
                if n == 0 and ct == 0:
                    xps.append(xp00)
                    continue
                xp = xp_pool.tile([128, HP, WP], F32R, tag="xp",
                                  name=f"xp{n}_{ct}")
                nc.sync.dma_start(out=xp, in_=x_v[n, ct])
                xps.append(xp)
            y_sb = y_pool.tile([128, 2, L], F32R, tag="y", name=f"y{n}")

            for ct in range(2):
                xp = xps[ct]
                N_PE_TAPS = PE_TAPS_BY_CT[ct]
                for r0, nr in CCHUNKS:
                    nb = nr // 8            # 8-row groups in this chunk (2 or 1)
                    cw = nr * W
                    ps = cps_pool.tile([128, 2, 512], F32, tag="cps",
                                       name=f"cps{n}_{ct}_{r0}")
                    for t in range(N_PE_TAPS):
                        dh, dw = divmod(t, 3)
                        for hb in range(nb):
                            nc.tensor.matmul(
                                ps[:, hb, 0:CONV_CHUNK],
                                lhsT=diag_sb[:, ct * 9 + t, :],
                                rhs=xp[:, r0 + 8 * hb + dh:r0 + 8 * hb + dh + 8,
                                       dw:dw + W],
                                start=(t == 0), stop=(t == N_PE_TAPS - 1),
                            )
                    ps_v = ps[:, 0:nb, 0:CONV_CHUNK].rearrange(
                        "p a (h w) -> p a h w", h=8)
                    for t in range(N_PE_TAPS, 9):
                        dh, dw = divmod(t, 3)
                        nc.vector.scalar_tensor_tensor(
                            out=ps_v,
                            in0=xp[:, r0 + dh:r0 + dh + nr,
                                   dw:dw + W].bitcast(F32).rearrange(
                                       "p (a h) w -> p a h w", a=nb),
                            scalar=w9c_sb[:, ct * 9 + t:ct * 9 + t + 1],
                            in1=ps_v, op0=mybir.AluOpType.mult, op1=ADD,
                        )
                    # epilogue: y = relu(ps + cbias[ct])
                    nc.scalar.activation(
                        out=y_sb[:, ct, r0 * W:r0 * W + cw].rearrange(
                            "p (a b) -> p a b", a=nb),
                        in_=ps[:, 0:nb, 0:CONV_CHUNK], func=RELU,
                        bias=cbias_sb[:, ct:ct + 1], scale=1.0,
                    )

            o_sb = o_pool.tile([128, 2, L], F32, tag="o", name=f"o{n}")
            for co in range(2):
                for s0, sl in BTF_CHUNKS:
                    bps = bps_pool.tile([128, 512], F32, tag="bps",
                                        name=f"bps{n}_{co}_{s0}")
                    for ci in range(2):
                        nc.tensor.matmul(
                            bps[:, :sl],
                            lhsT=mt_sb[:, ci * 2 + co, :],
                            rhs=y_sb[:, ci, s0:s0 + sl],
                            start=(ci == 0), stop=(ci == 1),
                        )
                    # butterfly epilogue: last image alternates DVE/ACT (tail
                    # latency); earlier images all-ACT (DVE is on conv taps)
                    if n == imgs - 1 and (s0 // 512) % 2 == 0:
                        nc.vector.tensor_scalar(
                            o_sb[:, co, s0:s0 + sl], bps[:, :sl],
                            obias_sb[:, co:co + 1], 0.0, ADD, MAX,
                        )
                    else:
                        nc.scalar.activation(
                            out=o_sb[:, co, s0:s0 + sl], in_=bps[:, :sl],
                            func=RELU, bias=obias_sb[:, co:co + 1], scale=1.0,
                        )
                if n == imgs - 1:
                    half = 1536
                    nc.sync.dma_start(out=out_v[n, :, co][:, :half],
                                      in_=o_sb[:, co, :half])
                    nc.sync.dma_start(out=out_v[n, :, co][:, half:],
                                      in_=o_sb[:, co, half:])
                else:
                    nc.scalar.dma_start(out=out_v[n, :, co], in_=o_sb[:, co])

    nc.compile()
    return nc


def _fold_params(dw_w, g1, b1, m1, v1, bw, bg, bb, bm, bv):
    """Fold BN1 into conv taps; fold butterfly+BN chain into (M, bias)."""
    f8 = np.float64
    dw_w, g1, b1, m1, v1 = (np.asarray(a, f8) for a in (dw_w, g1, b1, m1, v1))
    inv1 = g1 / np.sqrt(v1 + EPS)
    cbias = b1 - m1 * inv1                       # (256,)
    w9 = dw_w[:, 0] * inv1[:, None, None]        # (256, 3, 3)

    def chain(v):
        out = np.asarray(v, f8)[None, None]      # (1, 1, 256, cols)
        for wi, gi, bi_, mi, vi in zip(bw, bg, bb, bm, bv):
            wi, gi, bi_, mi, vi = (np.asarray(a, f8) for a in (wi, gi, bi_, mi, vi))
            g = out.shape[1]
            P = out.shape[2] // 2
            Lc = out.shape[3]
            x5 = out.reshape(1, g, P, 2, Lc)
            o = np.einsum("gkq,ngpql->ngkpl", wi, x5).reshape(1, 2 * g, P, Lc)
            inv = gi / np.sqrt(vi + EPS)
            out = o * inv[None, :, None, None] + (bi_ - mi * inv)[None, :, None, None]
        return out[0].reshape(256, -1)

    obias = chain(np.zeros((256, 1)))[:, 0]      # (256,)
    M = chain(np.eye(256)) - obias[:, None]      # (256, 256)

    # diag layout: [k, ct*9+t, p] (partition-major, contiguous DMA)
    diag = np.zeros((128, 18, 128), np.float32)
    k = np.arange(128)
    for ct in range(2):
        for t in range(9):
            dh, dw_ = divmod(t, 3)
            diag[k, ct * 9 + t, k] = w9[ct * 128 + k, dh, dw_].astype(np.float32)
    # mt layout: [k, ci*2+co, p] = M[co*128+p, ci*128+k]
    mt = np.zeros((128, 4, 128), np.float32)
    Mb = M.astype(np.float32).reshape(2, 128, 2, 128)   # [co, p, ci, k]
    for ci in range(2):
        for co in range(2):
            mt[:, ci * 2 + co, :] = Mb[co, :, ci, :].T
    w9c = np.zeros((128, 18), np.float32)
    for ct in range(2):
        for t in range(9):
            dh, dw_ = divmod(t, 3)
            w9c[:, ct * 9 + t] = w9[ct * 128:(ct + 1) * 128, dh, dw_].astype(np.float32)
    return (w9c, np.ascontiguousarray(diag), np.ascontiguousarray(mt),
            np.ascontiguousarray(cbias.reshape(2, 128).astype(np.float32).T),
            np.ascontiguousarray(obias.reshape(2, 128).astype(np.float32).T))


def _pad_x(x):
    """(N, 256, 56, 56) f32 -> (N, 2, 128, 58, 58) zero-ringed."""
    n = x.shape[0]
    xp = np.zeros((n, C, HP, WP), np.float32)
    xp[:, :, 1:57, 1:57] = x
    return np.ascontiguousarray(xp.reshape(n, 2, 128, HP, WP))


def make_in_maps(x, dw_w, g1, b1, m1, v1, bw, bg, bb, bm, bv):
    x = np.asarray(x, np.float32)
    w9c, diag, mt, cbias, obias = _fold_params(dw_w, g1, b1, m1, v1, bw, bg, bb, bm, bv)
    xpad = _pad_x(x)                              # (32, 2, 128, 58, 58)
    shards = xpad.reshape(N_CORES, IMGS, 2, 128, HP, WP)
    return [
        {"x": np.ascontiguousarray(shards[i]), "diag": diag, "mt": mt,
         "cbias": cbias, "obias": obias, "w9c": w9c}
        for i in range(N_CORES)
    ]


def kernel(x, dw_w, g1, b1, m1, v1, bw, bg, bb, bm, bv):
    in_maps = make_in_maps(x, dw_w, g1, b1, m1, v1, bw, bg, bb, bm, bv)
    if "nc" not in _compiled:
        _compiled["nc"] = _build()
    nc = _compiled["nc"]
    res = run_bass_kernel_spmd(nc, in_maps, core_ids=list(range(N_CORES)))
    out = np.concatenate([res.results[i]["out"] for i in range(N_CORES)], axis=0)
    return out.reshape(32, C, H, W)


# revision 23
# speedup vs baseline: 1.1322x; 1.1322x over previous
"""Trainium2 Bass kernel for nn_Block_56650618634972.

Math: reference = relu(AFFINE(relu(BN1(dwconv3x3(x)))))  where AFFINE is the
composition of 8 butterfly stages + per-stage BNs — all linear over the
256-channel axis — folded on host into a single 256x256 matrix M + bias.

Device work per core (batch-sharded, 4 images each):
  1. x arrives host-padded in a (58x58) zero-ringed layout, so every DMA is a
     full-rate contiguous transfer straight into SBUF.
  2. depthwise 3x3 conv: 9 diagonal-matrix matmuls (f32r) accumulating into
     PSUM, reading shifted windows of the padded tile.
  3. conv epilogue on ScalarE: relu(psum + beta1) -> y (SBUF, f32r)
  4. butterfly: dense 256x256 matmul (f32r, 2x2 blocks of 128)
  5. epilogue on VectorE: relu(psum + bias) -> out (SBUF, f32) -> DMA out

DMA queues: nc.sync carries the x-in stream; nc.scalar (the other HWDGE ring)
carries params + output stores (their issue cost on ACT is tiny).
"""

import numpy as np

import concourse.bass as bass
import concourse.mybir as mybir
import concourse.tile as tile
from concourse import bacc
from concourse.bass_utils import run_bass_kernel_spmd

N_CORES = 8
IMGS = 4            # images per core (32 / 8)
C = 256
H = W = 56
L = H * W           # 3136
HP = H + 2          # 58
WP = W + 2          # 58
PADL = HP * WP      # 3364
EPS = 1e-5
CONV_CHUNK_ROWS = 8                     # 8 rows x 56 = 448 cols <= 512 (1 PSUM bank)
CONV_CHUNK = CONV_CHUNK_ROWS * W        # 448
N_CONV_CHUNKS = H // CONV_CHUNK_ROWS    # 7
BTF_CHUNKS = [(i * 512, min(512, L - i * 512)) for i in range((L + 511) // 512)]

F32 = mybir.dt.float32
F32R = mybir.dt.float32r
RELU = mybir.ActivationFunctionType.Relu
ADD = mybir.AluOpType.add
MAX = mybir.AluOpType.max

_compiled = {}


def _build(imgs=IMGS):
    nc = bacc.Bacc("TRN2", target_bir_lowering=False, debug=False,
                   num_devices=N_CORES)
    x_d = nc.dram_tensor("x", (IMGS, 2, 128, HP, WP), F32R, kind="ExternalInput")
    diag_d = nc.dram_tensor("diag", (128, 18, 128), F32R, kind="ExternalInput")
    mt_d = nc.dram_tensor("mt", (128, 4, 128), F32R, kind="ExternalInput")
    cbias_d = nc.dram_tensor("cbias", (128, 2), F32, kind="ExternalInput")
    obias_d = nc.dram_tensor("obias", (128, 2), F32, kind="ExternalInput")
    w9c_d = nc.dram_tensor("w9c", (128, 18), F32, kind="ExternalInput")
    out_d = nc.dram_tensor("out", (IMGS, C, H, W), F32, kind="ExternalOutput")

    x_v = x_d.ap().rearrange("n t c a b -> n t c a b")
    out_v = out_d.ap().rearrange("n (t c) h w -> n c t (h w)", t=2)

    from contextlib import ExitStack
    with tile.TileContext(nc) as tc, ExitStack() as es:
        consts = es.enter_context(tc.tile_pool(name="consts", bufs=1))
        xp_pool = es.enter_context(tc.tile_pool(name="xppool", bufs=4))
        y_pool = es.enter_context(tc.tile_pool(name="ypool", bufs=2))
        o_pool = es.enter_context(tc.tile_pool(name="opool", bufs=2))
        cps_pool = es.enter_context(tc.tile_pool(name="cps", bufs=3, space="PSUM"))
        bps_pool = es.enter_context(tc.tile_pool(name="bps", bufs=2, space="PSUM"))

        # ---- constants: diag halves on sync ring; first xp piece + small
        # params on the scalar ring so both rings fill in parallel ----
        # sync ring: only first-matmul-critical bytes first; PE warm-up MMs
        # run on a zeroed scratch tile while these land, so HAM reaches 8/8
        # before real work starts
        diag_sb = consts.tile([128, 18, 128], F32R, name="diag_sb", tag="diag_sb")
        nc.sync.dma_start(out=diag_sb[:, 0:9], in_=diag_d.ap()[:, 0:9])
        xp00 = xp_pool.tile([128, HP, WP], F32R, tag="xp", name="xp0_0")
        nc.sync.dma_start(out=xp00[:, 0:35], in_=x_v[0, 0][:, 0:35])
        nc.sync.dma_start(out=xp00[:, 35:], in_=x_v[0, 0][:, 35:])
        nc.sync.dma_start(out=diag_sb[:, 9:18], in_=diag_d.ap()[:, 9:18])

        w9c_sb = consts.tile([128, 18], F32, name="w9c_sb", tag="w9c_sb")
        nc.scalar.dma_start(out=w9c_sb, in_=w9c_d.ap())
        cbias_sb = consts.tile([128, 2], F32, name="cbias_sb", tag="cbias_sb")
        nc.scalar.dma_start(out=cbias_sb, in_=cbias_d.ap())
        obias_sb = consts.tile([128, 2], F32, name="obias_sb", tag="obias_sb")
        nc.scalar.dma_start(out=obias_sb, in_=obias_d.ap())
        mt_sb = consts.tile([128, 4, 128], F32R, name="mt_sb", tag="mt_sb")
        nc.scalar.dma_start(out=mt_sb, in_=mt_d.ap())

        warm_sb = consts.tile([128, 128], F32R, name="warm_sb", tag="warm_sb")
        nc.vector.memset(warm_sb.bitcast(F32), 0.0)
        wps = bps_pool.tile([128, 512], F32, tag="bps", name="warm_ps")
        for wi in range(20):
            nc.tensor.matmul(wps, lhsT=warm_sb, rhs=warm_sb[:, 0:1].to_broadcast([128, 512]),
                             start=True, stop=True)

        # conv chunking: 16 rows (896 cols = 2 PSUM banks) + ragged last 8 rows
        CCHUNKS = [(0, 16), (16, 16), (32, 16), (48, 8)]
        PE_TAPS_BY_CT = (7, 7)

        for n in range(imgs):
            xps = []
            for ct in range(2):
                if n == 0 and ct == 0:
                    xps.append(xp00)
                    continue
                xp = xp_pool.tile([128, HP, WP], F32R, tag="xp",
                                  name=f"xp{n}_{ct}")
                nc.sync.dma_start(out=xp, in_=x_v[n, ct])
                xps.append(xp)
            y_sb = y_pool.tile([128, 2, L], F32R, tag="y", name=f"y{n}")

            for ct in range(2):
                xp = xps[ct]
                N_PE_TAPS = PE_TAPS_BY_CT[ct]
                for r0, nr in CCHUNKS:
                    nb = nr // 8            # 8-row groups in this chunk (2 or 1)
                    cw = nr * W
                    ps = cps_pool.tile([128, 2, 512], F32, tag="cps",
                                       name=f"cps{n}_{ct}_{r0}")
                    for t in range(N_PE_TAPS):
                        dh, dw = divmod(t, 3)
                        for hb in range(nb):
                            nc.tensor.matmul(
                                ps[:, hb, 0:CONV_CHUNK],
                                lhsT=diag_sb[:, ct * 9 + t, :],
                                rhs=xp[:, r0 + 8 * hb + dh:r0 + 8 * hb + dh + 8,
                                       dw:dw + W],
                                start=(t == 0), stop=(t == N_PE_TAPS - 1),
                            )
                    ps_v = ps[:, 0:nb, 0:CONV_CHUNK].rearrange(
                        "p a (h w) -> p a h w", h=8)
                    for t in range(N_PE_TAPS, 9):
                        dh, dw = divmod(t, 3)
                        nc.vector.scalar_tensor_tensor(
                            out=ps_v,
                            in0=xp[:, r0 + dh:r0 + dh + nr,
                                   dw:dw + W].bitcast(F32).rearrange(
                                       "p (a h) w -> p a h w", a=nb),
                            scalar=w9c_sb[:, ct * 9 + t:ct * 9 + t + 1],
                            in1=ps_v, op0=mybir.AluOpType.mult, op1=ADD,
                        )
                    # epilogue: y = relu(ps + cbias[ct])
                    nc.scalar.activation(
                        out=y_sb[:, ct, r0 * W:r0 * W + cw].rearrange(
                            "p (a b) -> p a b", a=nb),
                        in_=ps[:, 0:nb, 0:CONV_CHUNK], func=RELU,
                        bias=cbias_sb[:, ct:ct + 1], scale=1.0,
                    )

            o_sb = o_pool.tile([128, 2, L], F32, tag="o", name=f"o{n}")
            for co in range(2):
                for s0, sl in BTF_CHUNKS:
                    bps = bps_pool.tile([128, 512], F32, tag="bps",
                                        name=f"bps{n}_{co}_{s0}")
                    for ci in range(2):
                        nc.tensor.matmul(
                            bps[:, :sl],
                            lhsT=mt_sb[:, ci * 2 + co, :],
                            rhs=y_sb[:, ci, s0:s0 + sl],
                            start=(ci == 0), stop=(ci == 1),
                        )
                    # butterfly epilogue: last image alternates DVE/ACT (tail
                    # latency); earlier images all-ACT (DVE is on conv taps)
                    if n == imgs - 1 and (s0 // 512) % 2 == 0:
                        nc.vector.tensor_scalar(
                            o_sb[:, co, s0:s0 + sl], bps[:, :sl],
                            obias_sb[:, co:co + 1], 0.0, ADD, MAX,
                        )
                    else:
                        nc.scalar.activation(
                            out=o_sb[:, co, s0:s0 + sl], in_=bps[:, :sl],
                            func=RELU, bias=obias_sb[:, co:co + 1], scale=1.0,
                        )
                if n == imgs - 1:
                    half = 1536
                    nc.sync.dma_start(out=out_v[n, :, co][:, :half],
                                      in_=o_sb[:, co, :half])
                    nc.sync.dma_start(out=out_v[n, :, co][:, half:],
                                      in_=o_sb[:, co, half:])
                else:
                    nc.scalar.dma_start(out=out_v[n, :, co], in_=o_sb[:, co])

    nc.compile()
    return nc


def _fold_params(dw_w, g1, b1, m1, v1, bw, bg, bb, bm, bv):
    """Fold BN1 into conv taps; fold butterfly+BN chain into (M, bias)."""
    f8 = np.float64
    dw_w, g1, b1, m1, v1 = (np.asarray(a, f8) for a in (dw_w, g1, b1, m1, v1))
    inv1 = g1 / np.sqrt(v1 + EPS)
    cbias = b1 - m1 * inv1                       # (256,)
    w9 = dw_w[:, 0] * inv1[:, None, None]        # (256, 3, 3)

    def chain(v):
        out = np.asarray(v, f8)[None, None]      # (1, 1, 256, cols)
        for wi, gi, bi_, mi, vi in zip(bw, bg, bb, bm, bv):
            wi, gi, bi_, mi, vi = (np.asarray(a, f8) for a in (wi, gi, bi_, mi, vi))
            g = out.shape[1]
            P = out.shape[2] // 2
            Lc = out.shape[3]
            x5 = out.reshape(1, g, P, 2, Lc)
            o = np.einsum("gkq,ngpql->ngkpl", wi, x5).reshape(1, 2 * g, P, Lc)
            inv = gi / np.sqrt(vi + EPS)
            out = o * inv[None, :, None, None] + (bi_ - mi * inv)[None, :, None, None]
        return out[0].reshape(256, -1)

    obias = chain(np.zeros((256, 1)))[:, 0]      # (256,)
    M = chain(np.eye(256)) - obias[:, None]      # (256, 256)

    # diag layout: [k, ct*9+t, p] (partition-major, contiguous DMA)
    diag = np.zeros((128, 18, 128), np.float32)
    k = np.arange(128)
    for ct in range(2):
        for t in range(9):
            dh, dw_ = divmod(t, 3)
            diag[k, ct * 9 + t, k] = w9[ct * 128 + k, dh, dw_].astype(np.float32)
    # mt layout: [k, ci*2+co, p] = M[co*128+p, ci*128+k]
    mt = np.zeros((128, 4, 128), np.float32)
    Mb = M.astype(np.float32).reshape(2, 128, 2, 128)   # [co, p, ci, k]
    for ci in range(2):
        for co in range(2):
            mt[:, ci * 2 + co, :] = Mb[co, :, ci, :].T
    w9c = np.zeros((128, 18), np.float32)
    for ct in range(2):
        for t in range(9):
            dh, dw_ = divmod(t, 3)
            w9c[:, ct * 9 + t] = w9[ct * 128:(ct + 1) * 128, dh, dw_].astype(np.float32)
    return (w9c, np.ascontiguousarray(diag), np.ascontiguousarray(mt),
            np.ascontiguousarray(cbias.reshape(2, 128).astype(np.float32).T),
            np.ascontiguousarray(obias.reshape(2, 128).astype(np.float32).T))


def _pad_x(x):
    """(N, 256, 56, 56) f32 -> (N, 2, 128, 58, 58) zero-ringed."""
    n = x.shape[0]
    xp = np.zeros((n, C, HP, WP), np.float32)
    xp[:, :, 1:57, 1:57] = x
    return np.ascontiguousarray(xp.reshape(n, 2, 128, HP, WP))


def make_in_maps(x, dw_w, g1, b1, m1, v1, bw, bg, bb, bm, bv):
    x = np.asarray(x, np.float32)
    w9c, diag, mt, cbias, obias = _fold_params(dw_w, g1, b1, m1, v1, bw, bg, bb, bm, bv)
    xpad = _pad_x(x)                              # (32, 2, 128, 58, 58)
    shards = xpad.reshape(N_CORES, IMGS, 2, 128, HP, WP)
    return [
        {"x": np.ascontiguousarray(shards[i]), "diag": diag, "mt": mt,
         "cbias": cbias, "obias": obias, "w9c": w9c}
        for i in range(N_CORES)
    ]


def kernel(x, dw_w, g1, b1, m1, v1, bw, bg, bb, bm, bv):
    in_maps = make_in_maps(x, dw_w, g1, b1, m1, v1, bw, bg, bb, bm, bv)
    if "nc" not in _compiled:
        _compiled["nc"] = _build()
    nc = _compiled["nc"]
    res = run_bass_kernel_spmd(nc, in_maps, core_ids=list(range(N_CORES)))
    out = np.concatenate([res.results[i]["out"] for i in range(N_CORES)], axis=0)
    return out.reshape(32, C, H, W)


# revision 24
# speedup vs baseline: 1.1327x; 1.0004x over previous
"""Trainium2 Bass kernel for nn_Block_56650618634972.

Math: reference = relu(AFFINE(relu(BN1(dwconv3x3(x)))))  where AFFINE is the
composition of 8 butterfly stages + per-stage BNs — all linear over the
256-channel axis — folded on host into a single 256x256 matrix M + bias.

Device work per core (batch-sharded, 4 images each):
  1. x arrives host-padded in a (58x58) zero-ringed layout, so every DMA is a
     full-rate contiguous transfer straight into SBUF.
  2. depthwise 3x3 conv: 9 diagonal-matrix matmuls (f32r) accumulating into
     PSUM, reading shifted windows of the padded tile.
  3. conv epilogue on ScalarE: relu(psum + beta1) -> y (SBUF, f32r)
  4. butterfly: dense 256x256 matmul (f32r, 2x2 blocks of 128)
  5. epilogue on VectorE: relu(psum + bias) -> out (SBUF, f32) -> DMA out

DMA queues: nc.sync carries the x-in stream; nc.scalar (the other HWDGE ring)
carries params + output stores (their issue cost on ACT is tiny).
"""

import numpy as np

import concourse.bass as bass
import concourse.mybir as mybir
import concourse.tile as tile
from concourse import bacc
from concourse.bass_utils import run_bass_kernel_spmd

N_CORES = 8
IMGS = 4            # images per core (32 / 8)
C = 256
H = W = 56
L = H * W           # 3136
HP = H + 2          # 58
WP = W + 2          # 58
PADL = HP * WP      # 3364
EPS = 1e-5
CONV_CHUNK_ROWS = 8                     # 8 rows x 56 = 448 cols <= 512 (1 PSUM bank)
CONV_CHUNK = CONV_CHUNK_ROWS * W        # 448
N_CONV_CHUNKS = H // CONV_CHUNK_ROWS    # 7
BTF_CHUNKS = [(i * 512, min(512, L - i * 512)) for i in range((L + 511) // 512)]

F32 = mybir.dt.float32
F32R = mybir.dt.float32r
RELU = mybir.ActivationFunctionType.Relu
ADD = mybir.AluOpType.add
MAX = mybir.AluOpType.max

_compiled = {}


def _build(imgs=IMGS):
    nc = bacc.Bacc("TRN2", target_bir_lowering=False, debug=False,
                   num_devices=N_CORES)
    x_d = nc.dram_tensor("x", (IMGS, 2, 128, HP, WP), F32R, kind="ExternalInput")
    diag_d = nc.dram_tensor("diag", (128, 18, 128), F32R, kind="ExternalInput")
    mt_d = nc.dram_tensor("mt", (128, 4, 128), F32R, kind="ExternalInput")
    cbias_d = nc.dram_tensor("cbias", (128, 2), F32, kind="ExternalInput")
    obias_d = nc.dram_tensor("obias", (128, 2), F32, kind="ExternalInput")
    w9c_d = nc.dram_tensor("w9c", (128, 18), F32, kind="ExternalInput")
    out_d = nc.dram_tensor("out", (IMGS, C, H, W), F32, kind="ExternalOutput")

    x_v = x_d.ap().rearrange("n t c a b -> n t c a b")
    out_v = out_d.ap().rearrange("n (t c) h w -> n c t (h w)", t=2)

    from contextlib import ExitStack
    with tile.TileContext(nc) as tc, ExitStack() as es:
        consts = es.enter_context(tc.tile_pool(name="consts", bufs=1))
        xp_pool = es.enter_context(tc.tile_pool(name="xppool", bufs=4))
        y_pool = es.enter_context(tc.tile_pool(name="ypool", bufs=2))
        o_pool = es.enter_context(tc.tile_pool(name="opool", bufs=2))
        cps_pool = es.enter_context(tc.tile_pool(name="cps", bufs=3, space="PSUM"))
        bps_pool = es.enter_context(tc.tile_pool(name="bps", bufs=2, space="PSUM"))

        # ---- constants: diag halves on sync ring; first xp piece + small
        # params on the scalar ring so both rings fill in parallel ----
        # sync ring: only first-matmul-critical bytes first; PE warm-up MMs
        # run on a zeroed scratch tile while these land, so HAM reaches 8/8
        # before real work starts
        diag_sb = consts.tile([128, 18, 128], F32R, name="diag_sb", tag="diag_sb")
        nc.sync.dma_start(out=diag_sb[:, 0:9], in_=diag_d.ap()[:, 0:9])
        xp00 = xp_pool.tile([128, HP, WP], F32R, tag="xp", name="xp0_0")
        nc.sync.dma_start(out=xp00[:, 0:35], in_=x_v[0, 0][:, 0:35])
        nc.sync.dma_start(out=xp00[:, 35:], in_=x_v[0, 0][:, 35:])
        nc.sync.dma_start(out=diag_sb[:, 9:18], in_=diag_d.ap()[:, 9:18])

        w9c_sb = consts.tile([128, 18], F32, name="w9c_sb", tag="w9c_sb")
        nc.scalar.dma_start(out=w9c_sb, in_=w9c_d.ap())
        cbias_sb = consts.tile([128, 2], F32, name="cbias_sb", tag="cbias_sb")
        nc.scalar.dma_start(out=cbias_sb, in_=cbias_d.ap())
        obias_sb = consts.tile([128, 2], F32, name="obias_sb", tag="obias_sb")
        nc.scalar.dma_start(out=obias_sb, in_=obias_d.ap())
        mt_sb = consts.tile([128, 4, 128], F32R, name="mt_sb", tag="mt_sb")
        nc.scalar.dma_start(out=mt_sb, in_=mt_d.ap())

        warm_sb = consts.tile([128, 128], F32R, name="warm_sb", tag="warm_sb")
        nc.vector.memset(warm_sb.bitcast(F32), 0.0)
        wps = bps_pool.tile([128, 512], F32, tag="bps", name="warm_ps")
        for wi in range(20):
            nc.tensor.matmul(wps, lhsT=warm_sb, rhs=warm_sb[:, 0:1].to_broadcast([128, 512]),
                             start=True, stop=True)

        # conv chunking: 16 rows (896 cols = 2 PSUM banks) + ragged last 8 rows
        CCHUNKS = [(0, 16), (16, 16), (32, 16), (48, 8)]
        PE_TAPS_BY_CT = (7, 7)

        for n in range(imgs):
            xps = []
            for ct in range(2):
                if n == 0 and ct == 0:
                    xps.append(xp00)
                    continue
                xp = xp_pool.tile([128, HP, WP], F32R, tag="xp",
                                  name=f"xp{n}_{ct}")
                nc.sync.dma_start(out=xp, in_=x_v[n, ct])
                xps.append(xp)
            y_sb = y_pool.tile([128, 2, L], F32R, tag="y", name=f"y{n}")

            for ct in range(2):
                xp = xps[ct]
                N_PE_TAPS = PE_TAPS_BY_CT[ct]
                for r0, nr in CCHUNKS:
                    nb = nr // 8            # 8-row groups in this chunk (2 or 1)
                    cw = nr * W
                    ps = cps_pool.tile([128, 2, 512], F32, tag="cps",
                                       name=f"cps{n}_{ct}_{r0}")
                    for t in range(N_PE_TAPS):
                        dh, dw = divmod(t, 3)
                        for hb in range(nb):
                            nc.tensor.matmul(
                                ps[:, hb, 0:CONV_CHUNK],
                                lhsT=diag_sb[:, ct * 9 + t, :],
                                rhs=xp[:, r0 + 8 * hb + dh:r0 + 8 * hb + dh + 8,
                                       dw:dw + W],
                                start=(t == 0), stop=(t == N_PE_TAPS - 1),
                            )
                    ps_v = ps[:, 0:nb, 0:CONV_CHUNK].rearrange(
                        "p a (h w) -> p a h w", h=8)
                    for t in range(N_PE_TAPS, 9):
                        dh, dw = divmod(t, 3)
                        nc.vector.scalar_tensor_tensor(
                            out=ps_v,
                            in0=xp[:, r0 + dh:r0 + dh + nr,
                                   dw:dw + W].bitcast(F32).rearrange(
                                       "p (a h) w -> p a h w", a=nb),
                            scalar=w9c_sb[:, ct * 9 + t:ct * 9 + t + 1],
                            in1=ps_v, op0=mybir.AluOpType.mult, op1=ADD,
                        )
                    # epilogue: y = relu(ps + cbias[ct])
                    nc.scalar.activation(
                        out=y_sb[:, ct, r0 * W:r0 * W + cw].rearrange(
                            "p (a b) -> p a b", a=nb),
                        in_=ps[:, 0:nb, 0:CONV_CHUNK], func=RELU,
                        bias=cbias_sb[:, ct:ct + 1], scale=1.0,
                    )

            o_sb = o_pool.tile([128, 2, L], F32, tag="o", name=f"o{n}")
            for co in range(2):
                for ki, (s0, sl) in enumerate(BTF_CHUNKS):
                    # last image: conv psum pool is idle, rotate through it too
                    # so butterfly evictions never stall the PE
                    if n == imgs - 1 and ki % 2 == 1:
                        bps = cps_pool.tile([128, 2, 512], F32, tag="cps",
                                            name=f"bps{n}_{co}_{s0}")[:, 0]
                    else:
                        bps = bps_pool.tile([128, 512], F32, tag="bps",
                                            name=f"bps{n}_{co}_{s0}")
                    for ci in range(2):
                        nc.tensor.matmul(
                            bps[:, :sl],
                            lhsT=mt_sb[:, ci * 2 + co, :],
                            rhs=y_sb[:, ci, s0:s0 + sl],
                            start=(ci == 0), stop=(ci == 1),
                        )
                    # butterfly epilogue: last image alternates DVE/ACT (tail
                    # latency); earlier images all-ACT (DVE is on conv taps)
                    if n == imgs - 1 and (s0 // 512) % 2 == 0:
                        nc.vector.tensor_scalar(
                            o_sb[:, co, s0:s0 + sl], bps[:, :sl],
                            obias_sb[:, co:co + 1], 0.0, ADD, MAX,
                        )
                    else:
                        nc.scalar.activation(
                            out=o_sb[:, co, s0:s0 + sl], in_=bps[:, :sl],
                            func=RELU, bias=obias_sb[:, co:co + 1], scale=1.0,
                        )
                if n == imgs - 1:
                    for q0, q1 in ((0, 1024), (1024, 2048), (2048, 3136)):
                        nc.sync.dma_start(out=out_v[n, :, co][:, q0:q1],
                                          in_=o_sb[:, co, q0:q1])
                else:
                    nc.scalar.dma_start(out=out_v[n, :, co], in_=o_sb[:, co])

    nc.compile()
    return nc


def _fold_params(dw_w, g1, b1, m1, v1, bw, bg, bb, bm, bv):
    """Fold BN1 into conv taps; fold butterfly+BN chain into (M, bias)."""
    f8 = np.float64
    dw_w, g1, b1, m1, v1 = (np.asarray(a, f8) for a in (dw_w, g1, b1, m1, v1))
    inv1 = g1 / np.sqrt(v1 + EPS)
    cbias = b1 - m1 * inv1                       # (256,)
    w9 = dw_w[:, 0] * inv1[:, None, None]        # (256, 3, 3)

    def chain(v):
        out = np.asarray(v, f8)[None, None]      # (1, 1, 256, cols)
        for wi, gi, bi_, mi, vi in zip(bw, bg, bb, bm, bv):
            wi, gi, bi_, mi, vi = (np.asarray(a, f8) for a in (wi, gi, bi_, mi, vi))
            g = out.shape[1]
            P = out.shape[2] // 2
            Lc = out.shape[3]
            x5 = out.reshape(1, g, P, 2, Lc)
            o = np.einsum("gkq,ngpql->ngkpl", wi, x5).reshape(1, 2 * g, P, Lc)
            inv = gi / np.sqrt(vi + EPS)
            out = o * inv[None, :, None, None] + (bi_ - mi * inv)[None, :, None, None]
        return out[0].reshape(256, -1)

    obias = chain(np.zeros((256, 1)))[:, 0]      # (256,)
    M = chain(np.eye(256)) - obias[:, None]      # (256, 256)

    # diag layout: [k, ct*9+t, p] (partition-major, contiguous DMA)
    diag = np.zeros((128, 18, 128), np.float32)
    k = np.arange(128)
    for ct in range(2):
        for t in range(9):
            dh, dw_ = divmod(t, 3)
            diag[k, ct * 9 + t, k] = w9[ct * 128 + k, dh, dw_].astype(np.float32)
    # mt layout: [k, ci*2+co, p] = M[co*128+p, ci*128+k]
    mt = np.zeros((128, 4, 128), np.float32)
    Mb = M.astype(np.float32).reshape(2, 128, 2, 128)   # [co, p, ci, k]
    for ci in range(2):
        for co in range(2):
            mt[:, ci * 2 + co, :] = Mb[co, :, ci, :].T
    w9c = np.zeros((128, 18), np.float32)
    for ct in range(2):
        for t in range(9):
            dh, dw_ = divmod(t, 3)
            w9c[:, ct * 9 + t] = w9[ct * 128:(ct + 1) * 128, dh, dw_].astype(np.float32)
    return (w9c, np.ascontiguousarray(diag), np.ascontiguousarray(mt),
            np.ascontiguousarray(cbias.reshape(2, 128).astype(np.float32).T),
            np.ascontiguousarray(obias.reshape(2, 128).astype(np.float32).T))


def _pad_x(x):
    """(N, 256, 56, 56) f32 -> (N, 2, 128, 58, 58) zero-ringed."""
    n = x.shape[0]
    xp = np.zeros((n, C, HP, WP), np.float32)
    xp[:, :, 1:57, 1:57] = x
    return np.ascontiguousarray(xp.reshape(n, 2, 128, HP, WP))


def make_in_maps(x, dw_w, g1, b1, m1, v1, bw, bg, bb, bm, bv):
    x = np.asarray(x, np.float32)
    w9c, diag, mt, cbias, obias = _fold_params(dw_w, g1, b1, m1, v1, bw, bg, bb, bm, bv)
    xpad = _pad_x(x)                              # (32, 2, 128, 58, 58)
    shards = xpad.reshape(N_CORES, IMGS, 2, 128, HP, WP)
    return [
        {"x": np.ascontiguousarray(shards[i]), "diag": diag, "mt": mt,
         "cbias": cbias, "obias": obias, "w9c": w9c}
        for i in range(N_CORES)
    ]


def kernel(x, dw_w, g1, b1, m1, v1, bw, bg, bb, bm, bv):
    in_maps = make_in_maps(x, dw_w, g1, b1, m1, v1, bw, bg, bb, bm, bv)
    if "nc" not in _compiled:
        _compiled["nc"] = _build()
    nc = _compiled["nc"]
    res = run_bass_kernel_spmd(nc, in_maps, core_ids=list(range(N_CORES)))
    out = np.concatenate([res.results[i]["out"] for i in range(N_CORES)], axis=0)
    return out.reshape(32, C, H, W)


# revision 25
# speedup vs baseline: 1.1362x; 1.0032x over previous
"""Trainium2 Bass kernel for nn_Block_56650618634972.

Math: reference = relu(AFFINE(relu(BN1(dwconv3x3(x)))))  where AFFINE is the
composition of 8 butterfly stages + per-stage BNs — all linear over the
256-channel axis — folded on host into a single 256x256 matrix M + bias.

Device work per core (batch-sharded, 4 images each):
  1. x arrives host-padded in a (58x58) zero-ringed layout, so every DMA is a
     full-rate contiguous transfer straight into SBUF.
  2. depthwise 3x3 conv: 9 diagonal-matrix matmuls (f32r) accumulating into
     PSUM, reading shifted windows of the padded tile.
  3. conv epilogue on ScalarE: relu(psum + beta1) -> y (SBUF, f32r)
  4. butterfly: dense 256x256 matmul (f32r, 2x2 blocks of 128)
  5. epilogue on VectorE: relu(psum + bias) -> out (SBUF, f32) -> DMA out

DMA queues: nc.sync carries the x-in stream; nc.scalar (the other HWDGE ring)
carries params + output stores (their issue cost on ACT is tiny).
"""

import numpy as np

import concourse.bass as bass
import concourse.mybir as mybir
import concourse.tile as tile
from concourse import bacc
from concourse.bass_utils import run_bass_kernel_spmd

N_CORES = 8
IMGS = 4            # images per core (32 / 8)
C = 256
H = W = 56
L = H * W           # 3136
HP = H + 2          # 58
WP = W + 2          # 58
PADL = HP * WP      # 3364
EPS = 1e-5
CONV_CHUNK_ROWS = 8                     # 8 rows x 56 = 448 cols <= 512 (1 PSUM bank)
CONV_CHUNK = CONV_CHUNK_ROWS * W        # 448
N_CONV_CHUNKS = H // CONV_CHUNK_ROWS    # 7
BTF_CHUNKS = [(i * 512, min(512, L - i * 512)) for i in range((L + 511) // 512)]

F32 = mybir.dt.float32
F32R = mybir.dt.float32r
RELU = mybir.ActivationFunctionType.Relu
ADD = mybir.AluOpType.add
MAX = mybir.AluOpType.max

_compiled = {}


def _build(imgs=IMGS):
    nc = bacc.Bacc("TRN2", target_bir_lowering=False, debug=False,
                   num_devices=N_CORES)
    x_d = nc.dram_tensor("x", (IMGS, 2, 128, HP, WP), F32R, kind="ExternalInput")
    diag_d = nc.dram_tensor("diag", (128, 18, 128), F32R, kind="ExternalInput")
    mt_d = nc.dram_tensor("mt", (128, 4, 128), F32R, kind="ExternalInput")
    cbias_d = nc.dram_tensor("cbias", (128, 2), F32, kind="ExternalInput")
    obias_d = nc.dram_tensor("obias", (128, 2), F32, kind="ExternalInput")
    w9c_d = nc.dram_tensor("w9c", (128, 18), F32, kind="ExternalInput")
    out_d = nc.dram_tensor("out", (IMGS, C, H, W), F32, kind="ExternalOutput")

    x_v = x_d.ap().rearrange("n t c a b -> n t c a b")
    out_v = out_d.ap().rearrange("n (t c) h w -> n c t (h w)", t=2)

    from contextlib import ExitStack
    with tile.TileContext(nc) as tc, ExitStack() as es:
        consts = es.enter_context(tc.tile_pool(name="consts", bufs=1))
        xp_pool = es.enter_context(tc.tile_pool(name="xppool", bufs=4))
        y_pool = es.enter_context(tc.tile_pool(name="ypool", bufs=2))
        o_pool = es.enter_context(tc.tile_pool(name="opool", bufs=2))
        cps_pool = es.enter_context(tc.tile_pool(name="cps", bufs=3, space="PSUM"))
        bps_pool = es.enter_context(tc.tile_pool(name="bps", bufs=2, space="PSUM"))

        # ---- constants: diag halves on sync ring; first xp piece + small
        # params on the scalar ring so both rings fill in parallel ----
        # sync ring: only first-matmul-critical bytes first; PE warm-up MMs
        # run on a zeroed scratch tile while these land, so HAM reaches 8/8
        # before real work starts
        diag_sb = consts.tile([128, 18, 128], F32R, name="diag_sb", tag="diag_sb")
        nc.sync.dma_start(out=diag_sb[:, 0:9], in_=diag_d.ap()[:, 0:9])
        xp00 = xp_pool.tile([128, HP, WP], F32R, tag="xp", name="xp0_0")
        nc.sync.dma_start(out=xp00[:, 0:35], in_=x_v[0, 0][:, 0:35])
        nc.sync.dma_start(out=xp00[:, 35:], in_=x_v[0, 0][:, 35:])
        nc.sync.dma_start(out=diag_sb[:, 9:18], in_=diag_d.ap()[:, 9:18])

        w9c_sb = consts.tile([128, 18], F32, name="w9c_sb", tag="w9c_sb")
        nc.scalar.dma_start(out=w9c_sb, in_=w9c_d.ap())
        cbias_sb = consts.tile([128, 2], F32, name="cbias_sb", tag="cbias_sb")
        nc.scalar.dma_start(out=cbias_sb, in_=cbias_d.ap())
        obias_sb = consts.tile([128, 2], F32, name="obias_sb", tag="obias_sb")
        nc.scalar.dma_start(out=obias_sb, in_=obias_d.ap())
        mt_sb = consts.tile([128, 4, 128], F32R, name="mt_sb", tag="mt_sb")
        nc.scalar.dma_start(out=mt_sb, in_=mt_d.ap())

        warm_sb = consts.tile([128, 128], F32R, name="warm_sb", tag="warm_sb")
        nc.vector.memset(warm_sb.bitcast(F32), 0.0)
        wps = bps_pool.tile([128, 512], F32, tag="bps", name="warm_ps")
        for wi in range(20):
            nc.tensor.matmul(wps, lhsT=warm_sb, rhs=warm_sb[:, 0:1].to_broadcast([128, 512]),
                             start=True, stop=True)

        # conv chunking: 16 rows (896 cols = 2 PSUM banks) + ragged last 8 rows
        CCHUNKS = [(0, 16), (16, 16), (32, 16), (48, 8)]
        PE_TAPS_BY_CT = (7, 7)

        for n in range(imgs):
            xps = []
            for ct in range(2):
                if n == 0 and ct == 0:
                    xps.append(xp00)
                    continue
                xp = xp_pool.tile([128, HP, WP], F32R, tag="xp",
                                  name=f"xp{n}_{ct}")
                nc.sync.dma_start(out=xp, in_=x_v[n, ct])
                xps.append(xp)
            y_sb = y_pool.tile([128, 2, L], F32R, tag="y", name=f"y{n}")

            for ct in range(2):
                xp = xps[ct]
                N_PE_TAPS = PE_TAPS_BY_CT[ct]
                for r0, nr in CCHUNKS:
                    nb = nr // 8            # 8-row groups in this chunk (2 or 1)
                    cw = nr * W
                    ps = cps_pool.tile([128, 2, 512], F32, tag="cps",
                                       name=f"cps{n}_{ct}_{r0}")
                    for t in range(N_PE_TAPS):
                        dh, dw = divmod(t, 3)
                        for hb in range(nb):
                            nc.tensor.matmul(
                                ps[:, hb, 0:CONV_CHUNK],
                                lhsT=diag_sb[:, ct * 9 + t, :],
                                rhs=xp[:, r0 + 8 * hb + dh:r0 + 8 * hb + dh + 8,
                                       dw:dw + W],
                                start=(t == 0), stop=(t == N_PE_TAPS - 1),
                            )
                    ps_v = ps[:, 0:nb, 0:CONV_CHUNK].rearrange(
                        "p a (h w) -> p a h w", h=8)
                    for t in range(N_PE_TAPS, 9):
                        dh, dw = divmod(t, 3)
                        nc.vector.scalar_tensor_tensor(
                            out=ps_v,
                            in0=xp[:, r0 + dh:r0 + dh + nr,
                                   dw:dw + W].bitcast(F32).rearrange(
                                       "p (a h) w -> p a h w", a=nb),
                            scalar=w9c_sb[:, ct * 9 + t:ct * 9 + t + 1],
                            in1=ps_v, op0=mybir.AluOpType.mult, op1=ADD,
                        )
                    # epilogue: y = relu(ps + cbias[ct])
                    nc.scalar.activation(
                        out=y_sb[:, ct, r0 * W:r0 * W + cw].rearrange(
                            "p (a b) -> p a b", a=nb),
                        in_=ps[:, 0:nb, 0:CONV_CHUNK], func=RELU,
                        bias=cbias_sb[:, ct:ct + 1], scale=1.0,
                    )

            o_sb = o_pool.tile([128, 2, L], F32, tag="o", name=f"o{n}")
            if n < imgs - 1:
                for co in range(2):
                    for ki, (s0, sl) in enumerate(BTF_CHUNKS):
                        bps = bps_pool.tile([128, 512], F32, tag="bps",
                                            name=f"bps{n}_{co}_{s0}")
                        for ci in range(2):
                            nc.tensor.matmul(
                                bps[:, :sl],
                                lhsT=mt_sb[:, ci * 2 + co, :],
                                rhs=y_sb[:, ci, s0:s0 + sl],
                                start=(ci == 0), stop=(ci == 1),
                            )
                        nc.scalar.activation(
                            out=o_sb[:, co, s0:s0 + sl], in_=bps[:, :sl],
                            func=RELU, bias=obias_sb[:, co:co + 1], scale=1.0,
                        )
                    nc.scalar.dma_start(out=out_v[n, :, co], in_=o_sb[:, co])
            else:
                # last image: chunk-outer order, psum rotated through the idle
                # conv pool, evictions alternating engines, and per-chunk
                # output DMAs so the kernel tail is one small store
                for ki, (s0, sl) in enumerate(BTF_CHUNKS):
                    for co in range(2):
                        j = ki * 2 + co
                        if j % 2 == 1:
                            bps = cps_pool.tile([128, 2, 512], F32, tag="cps",
                                                name=f"bps{n}_{co}_{s0}")[:, 0]
                        else:
                            bps = bps_pool.tile([128, 512], F32, tag="bps",
                                                name=f"bps{n}_{co}_{s0}")
                        for ci in range(2):
                            nc.tensor.matmul(
                                bps[:, :sl],
                                lhsT=mt_sb[:, ci * 2 + co, :],
                                rhs=y_sb[:, ci, s0:s0 + sl],
                                start=(ci == 0), stop=(ci == 1),
                            )
                        if co == 0:
                            nc.vector.tensor_scalar(
                                o_sb[:, co, s0:s0 + sl], bps[:, :sl],
                                obias_sb[:, co:co + 1], 0.0, ADD, MAX,
                            )
                        else:
                            nc.scalar.activation(
                                out=o_sb[:, co, s0:s0 + sl], in_=bps[:, :sl],
                                func=RELU, bias=obias_sb[:, co:co + 1], scale=1.0,
                            )
                        nc.sync.dma_start(out=out_v[n, :, co][:, s0:s0 + sl],
                                          in_=o_sb[:, co, s0:s0 + sl])

    nc.compile()
    return nc


def _fold_params(dw_w, g1, b1, m1, v1, bw, bg, bb, bm, bv):
    """Fold BN1 into conv taps; fold butterfly+BN chain into (M, bias)."""
    f8 = np.float64
    dw_w, g1, b1, m1, v1 = (np.asarray(a, f8) for a in (dw_w, g1, b1, m1, v1))
    inv1 = g1 / np.sqrt(v1 + EPS)
    cbias = b1 - m1 * inv1                       # (256,)
    w9 = dw_w[:, 0] * inv1[:, None, None]        # (256, 3, 3)

    def chain(v):
        out = np.asarray(v, f8)[None, None]      # (1, 1, 256, cols)
        for wi, gi, bi_, mi, vi in zip(bw, bg, bb, bm, bv):
            wi, gi, bi_, mi, vi = (np.asarray(a, f8) for a in (wi, gi, bi_, mi, vi))
            g = out.shape[1]
            P = out.shape[2] // 2
            Lc = out.shape[3]
            x5 = out.reshape(1, g, P, 2, Lc)
            o = np.einsum("gkq,ngpql->ngkpl", wi, x5).reshape(1, 2 * g, P, Lc)
            inv = gi / np.sqrt(vi + EPS)
            out = o * inv[None, :, None, None] + (bi_ - mi * inv)[None, :, None, None]
        return out[0].reshape(256, -1)

    obias = chain(np.zeros((256, 1)))[:, 0]      # (256,)
    M = chain(np.eye(256)) - obias[:, None]      # (256, 256)

    # diag layout: [k, ct*9+t, p] (partition-major, contiguous DMA)
    diag = np.zeros((128, 18, 128), np.float32)
    k = np.arange(128)
    for ct in range(2):
        for t in range(9):
            dh, dw_ = divmod(t, 3)
            diag[k, ct * 9 + t, k] = w9[ct * 128 + k, dh, dw_].astype(np.float32)
    # mt layout: [k, ci*2+co, p] = M[co*128+p, ci*128+k]
    mt = np.zeros((128, 4, 128), np.float32)
    Mb = M.astype(np.float32).reshape(2, 128, 2, 128)   # [co, p, ci, k]
    for ci in range(2):
        for co in range(2):
            mt[:, ci * 2 + co, :] = Mb[co, :, ci, :].T
    w9c = np.zeros((128, 18), np.float32)
    for ct in range(2):
        for t in range(9):
            dh, dw_ = divmod(t, 3)
            w9c[:, ct * 9 + t] = w9[ct * 128:(ct + 1) * 128, dh, dw_].astype(np.float32)
    return (w9c, np.ascontiguousarray(diag), np.ascontiguousarray(mt),
            np.ascontiguousarray(cbias.reshape(2, 128).astype(np.float32).T),
            np.ascontiguousarray(obias.reshape(2, 128).astype(np.float32).T))


def _pad_x(x):
    """(N, 256, 56, 56) f32 -> (N, 2, 128, 58, 58) zero-ringed."""
    n = x.shape[0]
    xp = np.zeros((n, C, HP, WP), np.float32)
    xp[:, :, 1:57, 1:57] = x
    return np.ascontiguousarray(xp.reshape(n, 2, 128, HP, WP))


def make_in_maps(x, dw_w, g1, b1, m1, v1, bw, bg, bb, bm, bv):
    x = np.asarray(x, np.float32)
    w9c, diag, mt, cbias, obias = _fold_params(dw_w, g1, b1, m1, v1, bw, bg, bb, bm, bv)
    xpad = _pad_x(x)                              # (32, 2, 128, 58, 58)
    shards = xpad.reshape(N_CORES, IMGS, 2, 128, HP, WP)
    return [
        {"x": np.ascontiguousarray(shards[i]), "diag": diag, "mt": mt,
         "cbias": cbias, "obias": obias, "w9c": w9c}
        for i in range(N_CORES)
    ]


def kernel(x, dw_w, g1, b1, m1, v1, bw, bg, bb, bm, bv):
    in_maps = make_in_maps(x, dw_w, g1, b1, m1, v1, bw, bg, bb, bm, bv)
    if "nc" not in _compiled:
        _compiled["nc"] = _build()
    nc = _compiled["nc"]
    res = run_bass_kernel_spmd(nc, in_maps, core_ids=list(range(N_CORES)))
    out = np.concatenate([res.results[i]["out"] for i in range(N_CORES)], axis=0)
    return out.reshape(32, C, H, W)
